# revision 1
# baseline (speedup 1.0000x reference)
"""Trainium2 Bass kernel for nn_Attn: softmax(out_state @ (history @ W.T + b).T, axis=1).

Key algebra: E = out_state @ proj.T = (out_state @ W) @ history.T + (out_state @ b) 1^T.
The bias contributes a per-row constant, which softmax is invariant to, so it is
dropped entirely.  Per core (1/8 of out_state rows):
    A.T = W.T @ S.T        (PE, fp32r)
    E   = A @ H.T          (PE, fp32r; H transposed on-chip via PE transpose-mode)
    out = softmax(E, 1)    (DVE max/scale, ACT exp with per-row bias + sum accumulation)

fp32r operands are produced by SWDGE cast-DMAs (W, S, H, identity arrive
pre-rounded) or by the PSUM->SBUF copies that must happen anyway.
"""

import numpy as np

import concourse.bacc as bacc
import concourse.bass as bass
import concourse.tile as tile
from concourse import mybir
from concourse.bass_utils import run_bass_kernel_spmd

STATE, SEQ, HID = 4096, 8192, 1024
NCORES = 8
RPC = STATE // NCORES          # 512 out_state rows per core
ITILES = RPC // 128            # 4
KT = HID // 128                # 8 contraction tiles
SCHUNK = 512                   # seq columns per streamed chunk
NCHUNK = SEQ // SCHUNK         # 16
SSUB = SCHUNK // 128           # 4 row sub-tiles per chunk
NHALF = SEQ // 256             # H streamed in 256-row half-chunks

f32 = mybir.dt.float32
f32r = mybir.dt.float32r
AXX = mybir.AxisListType.X
EXP = mybir.ActivationFunctionType.Exp


def _build():
    nc = bacc.Bacc("TRN2", target_bir_lowering=False, debug=False)
    s_d = nc.dram_tensor("s", [RPC, HID], f32, kind="ExternalInput").ap()
    h_d = nc.dram_tensor("h", [SEQ, HID], f32, kind="ExternalInput").ap()
    w_d = nc.dram_tensor("w", [HID, HID], f32, kind="ExternalInput").ap()
    eye_d = nc.dram_tensor("eye", [128, 128], f32, kind="ExternalInput").ap()
    o_d = nc.dram_tensor("o", [RPC, SEQ], f32, kind="ExternalOutput").ap()

    with tile.TileContext(nc) as tc:
        with tc.tile_pool(name="persist", bufs=1) as persist, \
             tc.tile_pool(name="hraw", bufs=2) as hraw_p, \
             tc.tile_pool(name="htp", bufs=2) as ht_p, \
             tc.tile_pool(name="small", bufs=1) as small:

            # fp32r identity for transpose-mode matmuls
            ident = persist.tile([128, 128], f32r, name="ident")
            nc.gpsimd.dma_start(out=ident, in_=eye_d)
            # A.T laid out [k_partition, kt, i], fp32r (stationary operand of E matmuls)
            at_r = persist.tile([128, KT, RPC], f32r, name="at_r")

            half_tiles = {}

            def fetch_half(hh):
                hr = hraw_p.tile([128, 2, HID], f32r, name="hr")
                nc.gpsimd.dma_start(
                    out=hr,
                    in_=h_d[hh * 256:(hh + 1) * 256, :].rearrange(
                        "(a p) k -> p a k", p=128
                    ),
                )
                half_tiles[hh] = hr

            ht_tiles = {}
            hr_pair = {}
            tp_ps_cell = [None]

            def transpose_chunk_half(c, phase):
                """Transpose kb range [4*phase, 4*phase+4) of chunk c."""
                if phase == 0:
                    hr_pair[c] = (half_tiles.pop(2 * c), half_tiles.pop(2 * c + 1))
                    ht_tiles[c] = ht_p.tile([128, KT, SCHUNK], f32r, name="ht")
                hr0, hr1 = hr_pair[c]
                ht = ht_tiles[c]
                tp_ps = tp_ps_cell[0]
                for kb in range(4 * phase, 4 * phase + 4):
                    tp = tp_ps.tile([128, SCHUNK], f32r, name="tp")
                    for a in range(SSUB):
                        src = hr0 if a < 2 else hr1
                        nc.tensor.transpose(
                            tp[:, a * 128:(a + 1) * 128],
                            src[:, a % 2, kb * 128:(kb + 1) * 128],
                            ident,
                        )
                    # alternate PSUM->SBUF copies between DVE and ACT
                    if kb % 2 == 1:
                        nc.scalar.copy(out=ht[:, kb, :], in_=tp)
                    else:
                        nc.vector.tensor_copy(ht[:, kb, :], tp)
                if phase == 1:
                    hr_pair.pop(c)

            def transpose_chunk(c):
                transpose_chunk_half(c, 0)
                transpose_chunk_half(c, 1)

            # ---------------- Phase A: A.T = W.T @ S.T ----------------
            with tc.tile_pool(name="pa", bufs=1) as pa, \
                 tc.tile_pool(name="pa_ps", bufs=2, space="PSUM") as paps:
                s_r = pa.tile([128, ITILES, HID], f32r, name="s_r")
                nc.gpsimd.dma_start(out=s_r, in_=s_d.rearrange("(a p) k -> p a k", p=128))
                w_r = pa.tile([128, KT, HID], f32r, name="w_r")
                # split so A.T for the low kt half can start sooner
                nc.gpsimd.dma_start(
                    out=w_r[:, :, 0:HID // 2],
                    in_=w_d[:, 0:HID // 2].rearrange("(t p) k -> p t k", p=128),
                )
                nc.gpsimd.dma_start(
                    out=w_r[:, :, HID // 2:],
                    in_=w_d[:, HID // 2:].rearrange("(t p) k -> p t k", p=128),
                )
                fetch_half(0)
                fetch_half(1)

                # S.T via PE transposes: st_r[:, jt, :] = S.T[jt*128:(jt+1)*128, :]
                st_r = pa.tile([128, KT, RPC], f32r, name="st_r")
                for jt in range(KT):
                    ps = paps.tile([128, RPC], f32r, name="st_ps")
                    for a in range(ITILES):
                        nc.tensor.transpose(
                            ps[:, a * 128:(a + 1) * 128],
                            s_r[:, a, jt * 128:(jt + 1) * 128],
                            ident,
                        )
                    nc.vector.tensor_copy(st_r[:, jt, :], ps)

                # A.T[kt] = sum_jt W[jt, kt].T-block @ S.T[jt]
                for kt in range(KT):
                    ps = paps.tile([128, RPC], f32, name="at_ps")
                    for jt in range(KT):
                        nc.tensor.matmul(
                            ps,
                            w_r[:, jt, kt * 128:(kt + 1) * 128],
                            st_r[:, jt, :],
                            start=(jt == 0),
                            stop=(jt == KT - 1),
                        )
                    nc.vector.tensor_copy(at_r[:, kt, :], ps)

            # ---------------- Phase B: E = A @ H.T, streamed over seq chunks ----------------
            with tc.tile_pool(name="ebuf", bufs=1) as ebuf, \
                 tc.tile_pool(name="tp_ps", bufs=6, space="PSUM") as tp_ps, \
                 tc.tile_pool(name="e_ps", bufs=2, space="PSUM") as e_ps:
                tp_ps_cell[0] = tp_ps

                E = [ebuf.tile([128, SEQ], f32, name=f"E{i}") for i in range(ITILES)]
                strip = small.tile([128, ITILES, NCHUNK], f32, name="strip")
                ssum = small.tile([128, ITILES, NCHUNK], f32, name="ssum")

                # transpose chunk c+1 interleaved with chunk c's matmuls: each
                # half-block of transposes is followed by two E matmul groups,
                # so the PSUM->SBUF copies always have matmul-time slack and
                # neither PE nor the copy engines ever stall.
                fetch_half(2)
                fetch_half(3)
                transpose_chunk(0)

                def e_group(c, i, ht):
                    # E chunk accumulates in PSUM; the PSUM->SBUF copy IS the
                    # exp (ACT is rate-1 for any function): store
                    # P~ = exp(E - M_c) with M_c the chunk's own row max
                    # (strip holds -M_c), plus the chunk sum s_c.  The global
                    # correction exp(M_c - m)/l is applied in phase C.
                    eps = e_ps.tile([128, SCHUNK], f32, name="eps")
                    for kb in range(KT):
                        nc.tensor.matmul(
                            eps,
                            at_r[:, kb, i * 128:(i + 1) * 128],
                            ht[:, kb, :],
                            start=(kb == 0),
                            stop=(kb == KT - 1),
                        )
                    nc.vector.tensor_reduce(
                        out=strip[:, i, c:c + 1], in_=eps, axis=AXX,
                        op=mybir.AluOpType.max, negate=True,
                    )
                    nc.scalar.activation(
                        out=E[i][:, c * SCHUNK:(c + 1) * SCHUNK], in_=eps,
                        func=EXP, bias=strip[:, i, c:c + 1], scale=1.0,
                        accum_out=ssum[:, i, c:c + 1],
                    )

                for c in range(NCHUNK):
                    ht = ht_tiles.pop(c)
                    for phase in range(2):
                        if c + 1 < NCHUNK:
                            transpose_chunk_half(c + 1, phase)
                        for i in (0, 1) if phase == 0 else (2, 3):
                            e_group(c, i, ht)
                    for hh in (2 * c + 4, 2 * c + 5):
                        if hh < NHALF:
                            fetch_half(hh)

                # ---------------- Phase C: global softmax correction ----------------
                # E holds exp(E - M_c) per chunk; strip holds -M_c, ssum holds
                # s_c = sum exp(E - M_c).  m = max_c M_c, f_c = exp(M_c - m),
                # l = sum_c s_c f_c, and the final scale is g_c = f_c / l.
                negm = small.tile([128, ITILES], f32, name="negm")
                fbuf = small.tile([128, ITILES, NCHUNK], f32, name="fbuf")
                prod = small.tile([128, ITILES, NCHUNK], f32, name="prod")
                gbuf = small.tile([128, ITILES, NCHUNK], f32, name="gbuf")
                ltot = small.tile([128, ITILES], f32, name="ltot")
                linv = small.tile([128, ITILES], f32, name="linv")
                CPQ = 4  # chunks per output DMA
                for i in range(ITILES):
                    nc.vector.tensor_reduce(
                        out=negm[:, i:i + 1], in_=strip[:, i, :], axis=AXX,
                        op=mybir.AluOpType.min,
                    )
                    # f_c = exp(-strip_c + negm) = exp(M_c - m)
                    nc.scalar.activation(
                        out=fbuf[:, i, :], in_=strip[:, i, :], func=EXP,
                        bias=negm[:, i:i + 1], scale=-1.0,
                    )
                    nc.vector.tensor_tensor(
                        out=prod[:, i, :], in0=ssum[:, i, :], in1=fbuf[:, i, :],
                        op=mybir.AluOpType.mult,
                    )
                    nc.vector.reduce_sum(
                        out=ltot[:, i:i + 1], in_=prod[:, i, :], axis=AXX,
                    )
                    nc.vector.reciprocal(linv[:, i:i + 1], ltot[:, i:i + 1])
                    nc.vector.tensor_scalar_mul(
                        gbuf[:, i, :], fbuf[:, i, :], linv[:, i:i + 1],
                    )
                    for q in range(NCHUNK // CPQ):
                        for cc in range(CPQ * q, CPQ * q + CPQ):
                            sl = slice(cc * SCHUNK, (cc + 1) * SCHUNK)
                            if cc % 2 == 0:
                                nc.vector.tensor_scalar_mul(
                                    E[i][:, sl], E[i][:, sl], gbuf[:, i, cc:cc + 1],
                                )
                            else:
                                nc.scalar.activation(
                                    out=E[i][:, sl], in_=E[i][:, sl],
                                    func=mybir.ActivationFunctionType.Copy,
                                    bias=0.0, scale=gbuf[:, i, cc:cc + 1],
                                )
                        qsl = slice(q * CPQ * SCHUNK, (q + 1) * CPQ * SCHUNK)
                        nc.sync.dma_start(out=o_d[i * 128:(i + 1) * 128, qsl], in_=E[i][:, qsl])

    nc.compile()
    return nc


_NC = None


def _get_nc():
    global _NC
    if _NC is None:
        _NC = _build()
    return _NC


_EYE = np.eye(128, dtype=np.float32)


def _in_maps(out_state, history, W):
    out_state = np.ascontiguousarray(np.asarray(out_state, dtype=np.float32))
    history = np.ascontiguousarray(np.asarray(history, dtype=np.float32))
    W = np.ascontiguousarray(np.asarray(W, dtype=np.float32))
    return [
        {"s": out_state[c * RPC:(c + 1) * RPC], "h": history, "w": W, "eye": _EYE}
        for c in range(NCORES)
    ]


def kernel(out_state, history, W, b):
    nc = _get_nc()
    res = run_bass_kernel_spmd(nc, _in_maps(out_state, history, W), core_ids=list(range(NCORES)))
    return np.concatenate([res.results[c]["o"] for c in range(NCORES)], axis=0)



# revision 3
# speedup vs baseline: 1.3081x; 1.3081x over previous
"""Trainium2 Bass kernel for nn_Attn: softmax(out_state @ (history @ W.T + b).T, axis=1).

Key algebra: E = out_state @ proj.T = (out_state @ W) @ history.T + (out_state @ b) 1^T.
The bias contributes a per-row constant, which softmax is invariant to, so it is
dropped entirely.  Per core (1/8 of out_state rows):
    A.T = W.T @ S.T        (PE, fp16 operands, fp32 PSUM accumulate)
    E   = A @ H.T          (PE, fp16 operands, fp32 PSUM accumulate)
    out = softmax(E, 1)    (DVE per-chunk max, ACT exp with per-row bias + sum accum)

All transposes (S.T and H.T) are done by the DMA XBAR transpose engine directly
from DRAM (fp16), so the PE does only matmuls and the vector engines only the
softmax work.  H.T stays resident in SBUF (16MB fp16) so the four 128-row output
strips are processed strip-major: strip i's softmax normalization + fp32 output
stores overlap strip i+1's matmuls, leaving only the last strip's stores as tail.

Numerics: inputs are rounded to fp16 on the host (the XBAR requires a 2-byte
dtype in DRAM); accumulation is fp32 in PSUM.  Measured end-to-end rel err vs
the fp32 reference is ~2.6e-3 (gate 2e-2).  The softmax numerator is held as
fp16 exp(E - M_c) per 512-column chunk (M_c = chunk row-max); the final rescale
exp(M_c - m)/l is applied per chunk while converting to fp32 for the store.
"""

import numpy as np

import concourse.bacc as bacc
import concourse.bass as bass
import concourse.tile as tile
from concourse import mybir
from concourse.bass_utils import run_bass_kernel_spmd

STATE, SEQ, HID = 4096, 8192, 1024
NCORES = 8
RPC = STATE // NCORES          # 512 out_state rows per core
ITILES = RPC // 128            # 4 output strips per core
KT = HID // 128                # 8 contraction tiles
SCHUNK = 512                   # seq columns per chunk (one PSUM bank)
NCHUNK = SEQ // SCHUNK         # 16

f16 = mybir.dt.float16
f32 = mybir.dt.float32
AXX = mybir.AxisListType.X
EXP = mybir.ActivationFunctionType.Exp
COPY = mybir.ActivationFunctionType.Copy


def _build():
    nc = bacc.Bacc("TRN2", target_bir_lowering=False, debug=False)
    s_d = nc.dram_tensor("s", [RPC, HID], f16, kind="ExternalInput").ap()
    h_d = nc.dram_tensor("h", [SEQ, HID], f16, kind="ExternalInput").ap()
    w_d = nc.dram_tensor("w", [HID, HID], f16, kind="ExternalInput").ap()
    o_d = nc.dram_tensor("o", [RPC, SEQ], f32, kind="ExternalOutput").ap()

    with tile.TileContext(nc) as tc:
        with tc.tile_pool(name="persist", bufs=1) as persist, \
             tc.tile_pool(name="epool", bufs=2) as epool, \
             tc.tile_pool(name="stage", bufs=2) as stage_p, \
             tc.tile_pool(name="small", bufs=1) as small:

            # H.T resident: htall[p, kt, s] = H[s, kt*128+p]
            htall = persist.tile([128, KT, SEQ], f16, name="htall")
            # A.T: at_r[p, kt, i] = A.T[kt*128+p, i]
            at_r = persist.tile([128, KT, RPC], f16, name="at_r")

            strip = small.tile([128, ITILES, NCHUNK], f32, name="strip")  # -M_c
            ssum = small.tile([128, ITILES, NCHUNK], f32, name="ssum")    # s_c

            # ---------------- Phase A: A.T = W.T @ S.T ----------------
            with tc.tile_pool(name="pa", bufs=1) as pa, \
                 tc.tile_pool(name="pa_ps", bufs=2, space="PSUM") as paps:
                # S.T via XBAR: st[p, jt, i] = S[i, jt*128+p]
                st = pa.tile([128, KT, RPC], f16, name="st")
                nc.sync.dma_start(out=st, in_=s_d, transpose=True)
                # W halves so kt 0-3 can start after the first half lands
                w_r = pa.tile([128, KT, HID], f16, name="w_r")
                nc.sync.dma_start(
                    out=w_r[:, :, 0:HID // 2],
                    in_=w_d[:, 0:HID // 2].rearrange("(t p) k -> p t k", p=128),
                )
                nc.sync.dma_start(
                    out=w_r[:, :, HID // 2:],
                    in_=w_d[:, HID // 2:].rearrange("(t p) k -> p t k", p=128),
                )
                # H.T XBARs: queue all 16 chunks behind the phase-A loads
                for c in range(NCHUNK):
                    nc.sync.dma_start(
                        out=htall[:, :, c * SCHUNK:(c + 1) * SCHUNK],
                        in_=h_d[c * SCHUNK:(c + 1) * SCHUNK, :],
                        transpose=True,
                    )

                for kt in range(KT):
                    ps = paps.tile([128, RPC], f32, name="at_ps")
                    for jt in range(KT):
                        nc.tensor.matmul(
                            ps,
                            w_r[:, jt, kt * 128:(kt + 1) * 128],
                            st[:, jt, :],
                            start=(jt == 0),
                            stop=(jt == KT - 1),
                        )
                    nc.vector.tensor_copy(at_r[:, kt, :], ps)

            # ---------------- Phase B/C: E strips + softmax ----------------
            with tc.tile_pool(name="e_ps", bufs=3, space="PSUM") as e_ps:
                E = {}

                def e_group(i, c):
                    # E chunk accumulates in PSUM; the PSUM->SBUF copy IS the
                    # exp: store P~ = exp(E - M_c) (fp16) with M_c the chunk's
                    # own row max (strip holds -M_c), plus the chunk sum s_c.
                    eps = e_ps.tile([128, SCHUNK], f32, name="eps")
                    for kt in range(KT):
                        nc.tensor.matmul(
                            eps,
                            at_r[:, kt, i * 128:(i + 1) * 128],
                            htall[:, kt, c * SCHUNK:(c + 1) * SCHUNK],
                            start=(kt == 0),
                            stop=(kt == KT - 1),
                        )
                    nc.vector.tensor_reduce(
                        out=strip[:, i, c:c + 1], in_=eps, axis=AXX,
                        op=mybir.AluOpType.max, negate=True,
                    )
                    nc.scalar.activation(
                        out=E[i][:, c * SCHUNK:(c + 1) * SCHUNK], in_=eps,
                        func=EXP, bias=strip[:, i, c:c + 1], scale=1.0,
                        accum_out=ssum[:, i, c:c + 1],
                    )

                def finish_strip(i):
                    # m = max_c M_c, f_c = exp(M_c - m), l = sum_c s_c f_c,
                    # final per-chunk scale g_c = f_c / l applied during the
                    # fp16 -> fp32 staging copy, then stored.
                    negm = small.tile([128, 1], f32, name=f"negm{i}")
                    fbuf = small.tile([128, NCHUNK], f32, name=f"fbuf{i}")
                    prod = small.tile([128, NCHUNK], f32, name=f"prod{i}")
                    gbuf = small.tile([128, NCHUNK], f32, name=f"gbuf{i}")
                    ltot = small.tile([128, 1], f32, name=f"ltot{i}")
                    linv = small.tile([128, 1], f32, name=f"linv{i}")
                    nc.vector.tensor_reduce(
                        out=negm, in_=strip[:, i, :], axis=AXX,
                        op=mybir.AluOpType.min,
                    )
                    nc.scalar.activation(
                        out=fbuf, in_=strip[:, i, :], func=EXP,
                        bias=negm, scale=-1.0,
                    )
                    nc.vector.tensor_tensor(
                        out=prod, in0=ssum[:, i, :], in1=fbuf,
                        op=mybir.AluOpType.mult,
                    )
                    nc.vector.reduce_sum(out=ltot, in_=prod, axis=AXX)
                    nc.vector.reciprocal(linv, ltot)
                    nc.vector.tensor_scalar_mul(gbuf, fbuf, linv)
                    CPS = 2  # chunks per staging buffer / store
                    for q in range(NCHUNK // CPS):
                        stg = stage_p.tile([128, CPS * SCHUNK], f32, name="stg")
                        for cc in range(CPS * q, CPS * q + CPS):
                            ssl = slice((cc - CPS * q) * SCHUNK,
                                        (cc - CPS * q + 1) * SCHUNK)
                            esl = slice(cc * SCHUNK, (cc + 1) * SCHUNK)
                            if cc % 2 == 0:
                                nc.vector.tensor_scalar_mul(
                                    stg[:, ssl], E[i][:, esl], gbuf[:, cc:cc + 1],
                                )
                            else:
                                nc.scalar.activation(
                                    out=stg[:, ssl], in_=E[i][:, esl],
                                    func=COPY, bias=0.0, scale=gbuf[:, cc:cc + 1],
                                )
                        nc.sync.dma_start(
                            out=o_d[i * 128:(i + 1) * 128,
                                    q * CPS * SCHUNK:(q + 1) * CPS * SCHUNK],
                            in_=stg,
                        )

                # B1: strips 0,1 chunk-major (paced with XBAR arrivals),
                # then strips 2,3 strip-major over the resident H.T.
                E[0] = epool.tile([128, SEQ], f16, name="E")
                E[1] = epool.tile([128, SEQ], f16, name="E")
                for c in range(NCHUNK):
                    e_group(0, c)
                    e_group(1, c)
                finish_strip(0)
                E[2] = epool.tile([128, SEQ], f16, name="E")
                for c in range(NCHUNK):
                    e_group(2, c)
                finish_strip(1)
                E[3] = epool.tile([128, SEQ], f16, name="E")
                for c in range(NCHUNK):
                    e_group(3, c)
                finish_strip(2)
                finish_strip(3)

    nc.compile()
    return nc


_NC = None


def _get_nc():
    global _NC
    if _NC is None:
        _NC = _build()
    return _NC


def _in_maps(out_state, history, W):
    s16 = np.ascontiguousarray(np.asarray(out_state, dtype=np.float16))
    h16 = np.ascontiguousarray(np.asarray(history, dtype=np.float16))
    w16 = np.ascontiguousarray(np.asarray(W, dtype=np.float16))
    return [
        {"s": s16[c * RPC:(c + 1) * RPC], "h": h16, "w": w16}
        for c in range(NCORES)
    ]


def kernel(out_state, history, W, b):
    nc = _get_nc()
    res = run_bass_kernel_spmd(nc, _in_maps(out_state, history, W), core_ids=list(range(NCORES)))
    return np.concatenate([res.results[c]["o"] for c in range(NCORES)], axis=0)


# revision 9
# speedup vs baseline: 1.4123x; 1.0797x over previous
"""Trainium2 Bass kernel for nn_Attn: softmax(out_state @ (history @ W.T + b).T, axis=1).

Key algebra: E = out_state @ proj.T = (out_state @ W) @ history.T + (out_state @ b) 1^T.
The bias contributes a per-row constant, which softmax is invariant to, so it is
dropped entirely.  Per core (1/8 of out_state rows):
    A.T = W.T @ S.T        (PE, fp16 operands, fp32 PSUM accumulate)
    E   = A @ H.T          (PE, fp16 operands, fp32 PSUM accumulate)
    out = softmax(E, 1)    (DVE per-chunk max, ACT exp with per-row bias + sum accum)

All transposes (S.T and H.T) are done by the DMA XBAR transpose engine directly
from DRAM (fp16), so the PE does only matmuls and the vector engines only the
softmax work.  H.T stays resident in SBUF (16MB fp16) so the four 128-row output
strips are processed strip-major: strip i's softmax normalization + fp32 output
stores overlap strip i+1's matmuls, leaving only the last strip's stores as tail.

Numerics: inputs are rounded to fp16 on the host (the XBAR requires a 2-byte
dtype in DRAM); accumulation is fp32 in PSUM.  Measured end-to-end rel err vs
the fp32 reference is ~2.6e-3 (gate 2e-2).  The softmax numerator is held as
fp16 exp(E - M_c) per 512-column chunk (M_c = chunk row-max); the final rescale
exp(M_c - m)/l is applied per chunk while converting to fp32 for the store.
"""

import numpy as np

import concourse.bacc as bacc
import concourse.bass as bass
import concourse.tile as tile
from concourse import mybir
from concourse.bass_utils import run_bass_kernel_spmd

STATE, SEQ, HID = 4096, 8192, 1024
NCORES = 8
RPC = STATE // NCORES          # 512 out_state rows per core
ITILES = RPC // 128            # 4 output strips per core
KT = HID // 128                # 8 contraction tiles
SCHUNK = 512                   # seq columns per chunk (one PSUM bank)
NCHUNK = SEQ // SCHUNK         # 16

f16 = mybir.dt.float16
f32 = mybir.dt.float32
AXX = mybir.AxisListType.X
EXP = mybir.ActivationFunctionType.Exp
COPY = mybir.ActivationFunctionType.Copy


def _build():
    nc = bacc.Bacc("TRN2", target_bir_lowering=False, debug=False)
    s_d = nc.dram_tensor("s", [RPC, HID], f16, kind="ExternalInput").ap()
    h_d = nc.dram_tensor("h", [SEQ, HID], f16, kind="ExternalInput").ap()
    w_d = nc.dram_tensor("w", [HID, HID], f16, kind="ExternalInput").ap()
    o_d = nc.dram_tensor("o", [RPC, SEQ], f32, kind="ExternalOutput").ap()

    with tile.TileContext(nc) as tc:
        with tc.tile_pool(name="persist", bufs=1) as persist, \
             tc.tile_pool(name="small", bufs=1) as small:

            # H.T resident: htall[p, kt, s] = H[s, kt*128+p]
            htall = persist.tile([128, KT, SEQ], f16, name="htall")
            # A.T: at_r[p, kt, i] = A.T[kt*128+p, i]
            at_r = persist.tile([128, KT, RPC], f16, name="at_r")

            strip = small.tile([128, ITILES, NCHUNK], f32, name="strip")  # -M_c
            ssum = small.tile([128, ITILES, NCHUNK], f32, name="ssum")    # s_c

            # ---------------- Phase A: A.T = W.T @ S.T ----------------
            with tc.tile_pool(name="pa", bufs=1) as pa, \
                 tc.tile_pool(name="pa_ps", bufs=2, space="PSUM") as paps:
                # W first half, then S.T, then second half: A.T kt 0-3 can
                # start as soon as the first two transfers land.
                st = pa.tile([128, KT, RPC], f16, name="st")
                w_r = pa.tile([128, KT, HID], f16, name="w_r")
                nc.sync.dma_start(
                    out=w_r[:, :, 0:HID // 2],
                    in_=w_d[:, 0:HID // 2].rearrange("(t p) k -> p t k", p=128),
                )
                # S.T via XBAR: st[p, jt, i] = S[i, jt*128+p]
                nc.sync.dma_start(out=st, in_=s_d, transpose=True)
                nc.sync.dma_start(
                    out=w_r[:, :, HID // 2:],
                    in_=w_d[:, HID // 2:].rearrange("(t p) k -> p t k", p=128),
                )
                # H.T XBARs: queue all 16 chunks behind the phase-A loads
                for c in range(NCHUNK):
                    nc.sync.dma_start(
                        out=htall[:, :, c * SCHUNK:(c + 1) * SCHUNK],
                        in_=h_d[c * SCHUNK:(c + 1) * SCHUNK, :],
                        transpose=True,
                    )

                for kt in range(KT):
                    ps = paps.tile([128, RPC], f32, name="at_ps")
                    for jt in range(KT):
                        nc.tensor.matmul(
                            ps,
                            w_r[:, jt, kt * 128:(kt + 1) * 128],
                            st[:, jt, :],
                            start=(jt == 0),
                            stop=(jt == KT - 1),
                        )
                    nc.vector.tensor_copy(at_r[:, kt, :], ps)

            # ---------------- Phase B/C: E strips + softmax ----------------
            # (opened after the phase-A pool closes so its SBUF is reclaimed)
            with tc.tile_pool(name="epool", bufs=3) as epool, \
                 tc.tile_pool(name="stage", bufs=3) as stage_p, \
                 tc.tile_pool(name="e_ps", bufs=4, space="PSUM") as e_ps:
                E = {}

                def e_group(i, c):
                    # E chunk accumulates in PSUM; the PSUM->SBUF copy IS the
                    # exp: store P~ = exp(E - M_c) (fp16) with M_c the chunk's
                    # own row max (strip holds -M_c), plus the chunk sum s_c.
                    eps = e_ps.tile([128, SCHUNK], f32, name="eps")
                    for kt in range(KT):
                        nc.tensor.matmul(
                            eps,
                            at_r[:, kt, i * 128:(i + 1) * 128],
                            htall[:, kt, c * SCHUNK:(c + 1) * SCHUNK],
                            start=(kt == 0),
                            stop=(kt == KT - 1),
                        )
                    nc.vector.tensor_reduce(
                        out=strip[:, i, c:c + 1], in_=eps, axis=AXX,
                        op=mybir.AluOpType.max, negate=True,
                    )
                    nc.scalar.activation(
                        out=E[i][:, c * SCHUNK:(c + 1) * SCHUNK], in_=eps,
                        func=EXP, bias=strip[:, i, c:c + 1], scale=1.0,
                        accum_out=ssum[:, i, c:c + 1],
                    )

                def finish_strip(i):
                    # m = max_c M_c, f_c = exp(M_c - m), l = sum_c s_c f_c,
                    # final per-chunk scale g_c = f_c / l applied during the
                    # fp16 -> fp32 staging copy, then stored.
                    negm = small.tile([128, 1], f32, name=f"negm{i}")
                    fbuf = small.tile([128, NCHUNK], f32, name=f"fbuf{i}")
                    prod = small.tile([128, NCHUNK], f32, name=f"prod{i}")
                    gbuf = small.tile([128, NCHUNK], f32, name=f"gbuf{i}")
                    ltot = small.tile([128, 1], f32, name=f"ltot{i}")
                    linv = small.tile([128, 1], f32, name=f"linv{i}")
                    nc.vector.tensor_reduce(
                        out=negm, in_=strip[:, i, :], axis=AXX,
                        op=mybir.AluOpType.min,
                    )
                    nc.scalar.activation(
                        out=fbuf, in_=strip[:, i, :], func=EXP,
                        bias=negm, scale=-1.0,
                    )
                    nc.vector.tensor_tensor(
                        out=prod, in0=ssum[:, i, :], in1=fbuf,
                        op=mybir.AluOpType.mult,
                    )
                    nc.vector.reduce_sum(out=ltot, in_=prod, axis=AXX)
                    nc.vector.reciprocal(linv, ltot)
                    nc.vector.tensor_scalar_mul(gbuf, fbuf, linv)
                    for cc in range(NCHUNK):
                        stg = stage_p.tile([128, SCHUNK], f32, name="stg")
                        esl = slice(cc * SCHUNK, (cc + 1) * SCHUNK)
                        if cc % 2 == 0:
                            nc.vector.tensor_scalar_mul(
                                stg, E[i][:, esl], gbuf[:, cc:cc + 1],
                            )
                        else:
                            nc.scalar.activation(
                                out=stg, in_=E[i][:, esl],
                                func=COPY, bias=0.0, scale=gbuf[:, cc:cc + 1],
                            )
                        nc.sync.dma_start(
                            out=o_d[i * 128:(i + 1) * 128, esl], in_=stg,
                        )

                # B1: strips 0,1 chunk-major (paced with XBAR arrivals), then
                # strips 2,3 strip-major over the resident H.T.  Each strip's
                # softmax finish is emitted as early as its data allows so the
                # rescale + fp32 stores overlap later strips' matmuls; only
                # strip 3's finish is in the tail.
                E[0] = epool.tile([128, SEQ], f16, name="E")
                E[1] = epool.tile([128, SEQ], f16, name="E")
                for c in range(NCHUNK):
                    e_group(0, c)
                    e_group(1, c)
                finish_strip(0)
                E[2] = epool.tile([128, SEQ], f16, name="E")
                for c in range(NCHUNK // 2):
                    e_group(2, c)
                finish_strip(1)
                for c in range(NCHUNK // 2, NCHUNK):
                    e_group(2, c)
                finish_strip(2)
                E[3] = epool.tile([128, SEQ], f16, name="E")
                for c in range(NCHUNK):
                    e_group(3, c)
                finish_strip(3)

    nc.compile()
    return nc


_NC = None


def _get_nc():
    global _NC
    if _NC is None:
        _NC = _build()
    return _NC


def _in_maps(out_state, history, W):
    s16 = np.ascontiguousarray(np.asarray(out_state, dtype=np.float16))
    h16 = np.ascontiguousarray(np.asarray(history, dtype=np.float16))
    w16 = np.ascontiguousarray(np.asarray(W, dtype=np.float16))
    return [
        {"s": s16[c * RPC:(c + 1) * RPC], "h": h16, "w": w16}
        for c in range(NCORES)
    ]


def kernel(out_state, history, W, b):
    nc = _get_nc()
    res = run_bass_kernel_spmd(nc, _in_maps(out_state, history, W), core_ids=list(range(NCORES)))
    return np.concatenate([res.results[c]["o"] for c in range(NCORES)], axis=0)


# revision 10
# speedup vs baseline: 1.5131x; 1.0714x over previous
"""Trainium2 Bass kernel for nn_Attn: softmax(out_state @ (history @ W.T + b).T, axis=1).

Key algebra: E = out_state @ proj.T = (out_state @ W) @ history.T + (out_state @ b) 1^T.
The bias contributes a per-row constant, which softmax is invariant to, so it is
dropped entirely.  Per core (1/8 of out_state rows):
    A.T = W.T @ S.T        (PE, fp16 operands, fp32 PSUM accumulate)
    E   = A @ H.T          (PE, fp16 operands, fp32 PSUM accumulate)
    out = softmax(E, 1)    (DVE per-chunk max, ACT exp with per-row bias + sum accum)

All transposes (S.T and H.T) are done by the DMA XBAR transpose engine directly
from DRAM (fp16), so the PE does only matmuls and the vector engines only the
softmax work.  H.T stays resident in SBUF (16MB fp16) so the four 128-row output
strips are processed strip-major: strip i's softmax normalization + fp32 output
stores overlap strip i+1's matmuls, leaving only the last strip's stores as tail.

Numerics: inputs are rounded to fp16 on the host (the XBAR requires a 2-byte
dtype in DRAM); accumulation is fp32 in PSUM.  Measured end-to-end rel err vs
the fp32 reference is ~2.6e-3 (gate 2e-2).  The softmax numerator is held as
fp16 exp(E - M_c) per 512-column chunk (M_c = chunk row-max); the final rescale
exp(M_c - m)/l is applied per chunk while converting to fp32 for the store.
"""

import numpy as np

import concourse.bacc as bacc
import concourse.bass as bass
import concourse.tile as tile
from concourse import mybir
from concourse.bass_utils import run_bass_kernel_spmd

STATE, SEQ, HID = 4096, 8192, 1024
NCORES = 8
RPC = STATE // NCORES          # 512 out_state rows per core
ITILES = RPC // 128            # 4 output strips per core
KT = HID // 128                # 8 contraction tiles
SCHUNK = 512                   # seq columns per chunk (one PSUM bank)
NCHUNK = SEQ // SCHUNK         # 16

f16 = mybir.dt.float16
f32 = mybir.dt.float32
AXX = mybir.AxisListType.X
EXP = mybir.ActivationFunctionType.Exp
COPY = mybir.ActivationFunctionType.Copy


def _build():
    nc = bacc.Bacc("TRN2", target_bir_lowering=False, debug=False)
    s_d = nc.dram_tensor("s", [RPC, HID], f16, kind="ExternalInput").ap()
    h_d = nc.dram_tensor("h", [SEQ, HID], f16, kind="ExternalInput").ap()
    w_d = nc.dram_tensor("w", [HID, HID], f16, kind="ExternalInput").ap()
    o_d = nc.dram_tensor("o", [RPC, SEQ], f32, kind="ExternalOutput").ap()

    with tile.TileContext(nc) as tc:
        with tc.tile_pool(name="persist", bufs=1) as persist, \
             tc.tile_pool(name="small", bufs=1) as small:

            # H.T resident: htall[p, kt, s] = H[s, kt*128+p]
            htall = persist.tile([128, KT, SEQ], f16, name="htall")
            # A.T: at_r[p, kt, i] = A.T[kt*128+p, i]
            at_r = persist.tile([128, KT, RPC], f16, name="at_r")

            strip = small.tile([128, ITILES, NCHUNK], f32, name="strip")  # -M_c
            ssum = small.tile([128, ITILES, NCHUNK], f32, name="ssum")    # s_c

            # ---------------- Phase A: A.T = W.T @ S.T ----------------
            with tc.tile_pool(name="pa", bufs=1) as pa, \
                 tc.tile_pool(name="pa_ps", bufs=2, space="PSUM") as paps:
                # W first half, then S.T, then second half: A.T kt 0-3 can
                # start as soon as the first two transfers land.
                st = pa.tile([128, KT, RPC], f16, name="st")
                w_r = pa.tile([128, KT, HID], f16, name="w_r")
                # S.T via XBAR: st[p, jt, i] = S[i, jt*128+p]
                nc.sync.dma_start(out=st, in_=s_d, transpose=True)
                nc.sync.dma_start(
                    out=w_r[:, :, 0:HID // 2],
                    in_=w_d[:, 0:HID // 2].rearrange("(t p) k -> p t k", p=128),
                )
                nc.sync.dma_start(
                    out=w_r[:, :, HID // 2:],
                    in_=w_d[:, HID // 2:].rearrange("(t p) k -> p t k", p=128),
                )
                # H.T XBARs: queue all 16 chunks behind the phase-A loads
                for c in range(NCHUNK):
                    nc.sync.dma_start(
                        out=htall[:, :, c * SCHUNK:(c + 1) * SCHUNK],
                        in_=h_d[c * SCHUNK:(c + 1) * SCHUNK, :],
                        transpose=True,
                    )

                for kt in range(KT):
                    ps = paps.tile([128, RPC], f32, name="at_ps")
                    for jt in range(KT):
                        nc.tensor.matmul(
                            ps,
                            w_r[:, jt, kt * 128:(kt + 1) * 128],
                            st[:, jt, :],
                            start=(jt == 0),
                            stop=(jt == KT - 1),
                        )
                    nc.vector.tensor_copy(at_r[:, kt, :], ps)

            # ---------------- Phase B/C: E strips + softmax ----------------
            # (opened after the phase-A pool closes so its SBUF is reclaimed)
            with tc.tile_pool(name="epool", bufs=3) as epool, \
                 tc.tile_pool(name="stage", bufs=4) as stage_p, \
                 tc.tile_pool(name="e_ps", bufs=6, space="PSUM") as e_ps:
                E = {}

                def e_group(i, c):
                    # E chunk accumulates in PSUM; the PSUM->SBUF copy IS the
                    # exp: store P~ = exp(E - M_c) (fp16) with M_c the chunk's
                    # own row max (strip holds -M_c), plus the chunk sum s_c.
                    eps = e_ps.tile([128, SCHUNK], f32, name="eps")
                    for kt in range(KT):
                        nc.tensor.matmul(
                            eps,
                            at_r[:, kt, i * 128:(i + 1) * 128],
                            htall[:, kt, c * SCHUNK:(c + 1) * SCHUNK],
                            start=(kt == 0),
                            stop=(kt == KT - 1),
                        )
                    nc.vector.tensor_reduce(
                        out=strip[:, i, c:c + 1], in_=eps, axis=AXX,
                        op=mybir.AluOpType.max, negate=True,
                    )
                    nc.scalar.activation(
                        out=E[i][:, c * SCHUNK:(c + 1) * SCHUNK], in_=eps,
                        func=EXP, bias=strip[:, i, c:c + 1], scale=1.0,
                        accum_out=ssum[:, i, c:c + 1],
                    )

                def finish_strip(i):
                    # m = max_c M_c, f_c = exp(M_c - m), l = sum_c s_c f_c,
                    # final per-chunk scale g_c = f_c / l applied during the
                    # fp16 -> fp32 staging copy, then stored.
                    negm = small.tile([128, 1], f32, name=f"negm{i}")
                    fbuf = small.tile([128, NCHUNK], f32, name=f"fbuf{i}")
                    prod = small.tile([128, NCHUNK], f32, name=f"prod{i}")
                    gbuf = small.tile([128, NCHUNK], f32, name=f"gbuf{i}")
                    ltot = small.tile([128, 1], f32, name=f"ltot{i}")
                    linv = small.tile([128, 1], f32, name=f"linv{i}")
                    nc.vector.tensor_reduce(
                        out=negm, in_=strip[:, i, :], axis=AXX,
                        op=mybir.AluOpType.min,
                    )
                    nc.scalar.activation(
                        out=fbuf, in_=strip[:, i, :], func=EXP,
                        bias=negm, scale=-1.0,
                    )
                    nc.vector.tensor_tensor(
                        out=prod, in0=ssum[:, i, :], in1=fbuf,
                        op=mybir.AluOpType.mult,
                    )
                    nc.vector.reduce_sum(out=ltot, in_=prod, axis=AXX)
                    nc.vector.reciprocal(linv, ltot)
                    nc.vector.tensor_scalar_mul(gbuf, fbuf, linv)
                    CPS = 2  # chunks per staging buffer / store
                    for q in range(NCHUNK // CPS):
                        stg = stage_p.tile([128, CPS * SCHUNK], f32, name="stg")
                        for cc in range(CPS * q, CPS * q + CPS):
                            ssl = slice((cc - CPS * q) * SCHUNK,
                                        (cc - CPS * q + 1) * SCHUNK)
                            esl = slice(cc * SCHUNK, (cc + 1) * SCHUNK)
                            if cc % 2 == 0:
                                nc.vector.tensor_scalar_mul(
                                    stg[:, ssl], E[i][:, esl], gbuf[:, cc:cc + 1],
                                )
                            else:
                                nc.scalar.activation(
                                    out=stg[:, ssl], in_=E[i][:, esl],
                                    func=COPY, bias=0.0, scale=gbuf[:, cc:cc + 1],
                                )
                        nc.sync.dma_start(
                            out=o_d[i * 128:(i + 1) * 128,
                                    q * CPS * SCHUNK:(q + 1) * CPS * SCHUNK],
                            in_=stg,
                        )

                # B1: strips 0,1 chunk-major (paced with XBAR arrivals), then
                # strips 2,3 strip-major over the resident H.T.  Each strip's
                # softmax finish is emitted as early as its data allows so the
                # rescale + fp32 stores overlap later strips' matmuls; only
                # strip 3's finish is in the tail.
                E[0] = epool.tile([128, SEQ], f16, name="E")
                E[1] = epool.tile([128, SEQ], f16, name="E")
                for c in range(NCHUNK):
                    e_group(0, c)
                    e_group(1, c)
                finish_strip(0)
                E[2] = epool.tile([128, SEQ], f16, name="E")
                for c in range(NCHUNK // 2):
                    e_group(2, c)
                finish_strip(1)
                for c in range(NCHUNK // 2, NCHUNK):
                    e_group(2, c)
                finish_strip(2)
                E[3] = epool.tile([128, SEQ], f16, name="E")
                for c in range(NCHUNK):
                    e_group(3, c)
                finish_strip(3)

    nc.compile()
    return nc


_NC = None


def _get_nc():
    global _NC
    if _NC is None:
        _NC = _build()
    return _NC


def _in_maps(out_state, history, W):
    s16 = np.ascontiguousarray(np.asarray(out_state, dtype=np.float16))
    h16 = np.ascontiguousarray(np.asarray(history, dtype=np.float16))
    w16 = np.ascontiguousarray(np.asarray(W, dtype=np.float16))
    return [
        {"s": s16[c * RPC:(c + 1) * RPC], "h": h16, "w": w16}
        for c in range(NCORES)
    ]


def kernel(out_state, history, W, b):
    nc = _get_nc()
    res = run_bass_kernel_spmd(nc, _in_maps(out_state, history, W), core_ids=list(range(NCORES)))
    return np.concatenate([res.results[c]["o"] for c in range(NCORES)], axis=0)
